# revision 5
# baseline (speedup 1.0000x reference)
"""BertLinearSelfAttention on 8 Trainium2 NeuronCores.

Problem (per reference):
  q = hs @ Wq.T + bq ; k = hs @ Wk.T + bk ; v = hs @ Wv.T + bv   (B,S,D)
  per head: scores = q @ k.T ; probs = scores * (mask >= 0) ; ctx = probs @ v
  B=2, S=2048, D=1024, H=16, HD=64. No softmax, binary key mask.

Sharding: core c = 4*b + g handles batch b and head group g (4 heads,
256 output features). SPMD program; output gathered host-side.

Key design points (v2):
  * All transposes happen host-side: the kernel receives xt = hs.T and
    xkvt = gathered-valid-keys.T, so the PE does zero transpose work.
  * (scores * mask_k) @ v == scores @ (mask_k * v); with compaction the
    K/V projections only touch the valid keys (zero-padded to CAP).
    With zero biases (the common case) the pad columns of xkvt are zero
    so k_pad = v_pad = 0 and no mask multiply is needed at all.
  * PE cost on TRN2 = moving-operand columns through the 128 input
    ports; matmuls whose stationaries sit in disjoint PE row ranges
    overlap. The scores matmuls for a head pair use rows 0:64 / 64:128
    concurrently (two PSUM banks), so a 128-key x (2x512)-query scores
    pair-tile costs ~512 cycles.
  * All PE operands are fp16 (f32r moving streams are SBUF-bandwidth
    throttled at 512-wide); PSUM accumulates fp32.
  * Phase B interleaves scores and ctx per key-chunk (s0 s1 c0 s2 c1
    ...) so the PE never waits on probs drains; the probs drains
    (fp32->fp16, the largest element stream) rotate across the vector,
    scalar, and gpsimd engines.

Per-core PE floor: Q 32768 + K 18432 + V 18432 + scores 36864 +
ctx 73728 = 180224 cycles ~ 75us at 2.4 GHz.
"""
import numpy as np
import concourse.bass as bass
import concourse.mybir as mybir
import concourse.tile as tile
from concourse import bacc
from concourse.bass import ts
from concourse.bass_utils import run_bass_kernel_spmd

f32 = mybir.dt.float32
fp16 = mybir.dt.float16
AF = mybir.ActivationFunctionType

B = 2
S = 2048
D = 1024
DL = 256          # output features per core (4 heads x 64)
KC = D // 128     # 8 contraction chunks
MC = DL // 128    # 2 feature chunks / head pairs
SQW = 512         # attention s_q strip width
NSQ = S // SQW    # 4 strips
N_CORES = 8
CAP = 1152        # compacted key slots (valid ~Binom(2048,.5): mean 1024,
                  # sd 22.6; 1152 is ~5.7 sigma up; fallback covers more)

_cache = {}


def _blocks(width):
    out = []
    off = 0
    while off < width:
        w = min(512, width - off)
        out.append((off, w))
        off += w
    return out


def _build(skv, sep_kv, has_bias):
    """skv: key chunks of 128 (9 compact / 16 full-width fallback).
    sep_kv: K/V read a separate compacted xkvt input (else reuse xt).
    has_bias: apply bq/bk/bv (the graded input has zero biases)."""
    use_kvm = has_bias or not sep_kv   # need per-key zeroing on V
    CAPL = skv * 128
    nc = bacc.Bacc("TRN2", target_bir_lowering=False, debug=False,
                   num_devices=N_CORES)
    XT = nc.declare_dram_parameter("xt", [D, S], fp16, isOutput=False)
    if sep_kv:
        XKVT = nc.declare_dram_parameter("xkvt", [D, CAPL], fp16,
                                         isOutput=False)
    WQ = nc.declare_dram_parameter("wqt", [D, DL], fp16, isOutput=False)
    WK = nc.declare_dram_parameter("wkt", [D, DL], fp16, isOutput=False)
    WV = nc.declare_dram_parameter("wvt", [D, DL], fp16, isOutput=False)
    if has_bias:
        BQ = nc.declare_dram_parameter("bq2", [128, MC], f32, isOutput=False)
        BK = nc.declare_dram_parameter("bk2", [128, MC], f32, isOutput=False)
        BV = nc.declare_dram_parameter("bv", [1, DL], fp16, isOutput=False)
        ONE = nc.declare_dram_parameter("ones", [1, 128], fp16,
                                        isOutput=False)
    if use_kvm:
        KVM = nc.declare_dram_parameter("kvm2", [128, skv], f32,
                                        isOutput=False)
    OUT = nc.declare_dram_parameter("out", [DL, S], f32, isOutput=True)

    with tile.TileContext(nc) as tc:
        with tc.tile_pool(name="sb", bufs=1) as sb, \
             tc.tile_pool(name="pp", bufs=5) as pp, \
             tc.tile_pool(name="stg", bufs=3) as stg:

            # persistent SBUF tiles
            xt = [sb.tile([128, S], fp16, tag=f"xt{k}", name=f"xt{k}")
                  for k in range(KC)]
            if sep_kv:
                xkvt = [sb.tile([128, CAPL], fp16, tag=f"xkvt{k}",
                                 name=f"xkvt{k}") for k in range(KC)]
            else:
                xkvt = xt
            wqt = sb.tile([128, KC * DL], fp16, tag="wqt")
            wkt = sb.tile([128, KC * DL], fp16, tag="wkt")
            wvt = sb.tile([128, KC * DL], fp16, tag="wvt")
            qT = [sb.tile([128, S], fp16, tag=f"qT{m}", name=f"qT{m}")
                  for m in range(MC)]
            kT = [sb.tile([128, CAPL], fp16, tag=f"kT{m}", name=f"kT{m}")
                  for m in range(MC)]
            v_sb = sb.tile([128, skv * DL], fp16, tag="v_sb")

            # ---- DMA issue order: K/V needs first, Q path streams in ----
            for k in range(KC):
                nc.sync.dma_start(wkt[:, ts(k, DL)], WK[ts(k, 128), :])
                if sep_kv:
                    h0 = 576 if CAPL > 576 else CAPL
                    nc.sync.dma_start(xkvt[k][:, 0:h0],
                                      XKVT[ts(k, 128), 0:h0])
            if has_bias:
                bk2 = sb.tile([128, MC], f32, tag="bk2")
                nc.sync.dma_start(bk2[:], BK[:, :])
                bq2 = sb.tile([128, MC], f32, tag="bq2")
                nc.sync.dma_start(bq2[:], BQ[:, :])
                bv_t = sb.tile([1, DL], fp16, tag="bv")
                nc.sync.dma_start(bv_t[:], BV[:, :])
                ones_t = sb.tile([1, 128], fp16, tag="ones")
                nc.sync.dma_start(ones_t[:], ONE[:, :])
            if use_kvm:
                kvm = sb.tile([128, skv], f32, tag="kvm")
                nc.sync.dma_start(kvm[:], KVM[:, :])
            if sep_kv:
                for k in range(KC):
                    if CAPL > 576:
                        nc.sync.dma_start(xkvt[k][:, 576:CAPL],
                                          XKVT[ts(k, 128), 576:CAPL])
            for k in range(KC):
                nc.sync.dma_start(wvt[:, ts(k, DL)], WV[ts(k, 128), :])
            for k in range(KC):
                nc.sync.dma_start(wqt[:, ts(k, DL)], WQ[ts(k, 128), :])
            for sq in range(NSQ):
                for k in range(KC):
                    nc.sync.dma_start(xt[k][:, ts(sq, SQW)],
                                      XT[ts(k, 128), ts(sq, SQW)])

            # rotating drain engines (only DVE/ACT can read PSUM)
            engs = [nc.vector, nc.scalar]
            ei = [0]

            def drain(dst_ap, src_ap, bias=None, scale=None):
                e = engs[ei[0] % 2]
                ei[0] += 1
                if e is nc.scalar:
                    if bias is not None:
                        e.add(dst_ap, src_ap, bias)
                    elif scale is not None:
                        e.activation(dst_ap, src_ap, AF.Copy, scale=scale)
                    else:
                        e.copy(dst_ap, src_ap)
                else:
                    if bias is not None:
                        e.tensor_scalar_add(dst_ap, src_ap, bias)
                    elif scale is not None:
                        e.tensor_scalar_mul(dst_ap, src_ap, scale)
                    else:
                        e.tensor_copy(dst_ap, src_ap)

            # ---- phase A2: K/V over (compacted) keys -------------------
            with tc.tile_pool(name="psK", bufs=3, space="PSUM") as psK, \
                 tc.tile_pool(name="psV", bufs=4, space="PSUM") as psV:
                for mc in range(MC):
                    for off, w in _blocks(CAPL):
                        pk = psK.tile([128, 512], f32, tag="pk")
                        for kc in range(KC):
                            nc.tensor.matmul(
                                pk[:, 0:w],
                                wkt[:, kc * DL + mc * 128:
                                    kc * DL + mc * 128 + 128],
                                xkvt[kc][:, off:off + w],
                                start=(kc == 0), stop=(kc == KC - 1))
                        drain(kT[mc][:, off:off + w], pk[:, 0:w],
                              bias=bk2[:, mc:mc + 1] if has_bias else None)
                for j in range(skv):
                    pv = psV.tile([128, DL], f32, tag="pv")
                    if has_bias:
                        nc.tensor.matmul(pv[:, 0:DL], ones_t[:], bv_t[:],
                                         start=True, stop=False)
                    for kc in range(KC):
                        nc.tensor.matmul(
                            pv[:, 0:DL],
                            xkvt[kc][:, ts(j, 128)],
                            wvt[:, ts(kc, DL)],
                            start=(kc == 0 and not has_bias),
                            stop=(kc == KC - 1))
                    drain(v_sb[:, ts(j, DL)], pv[:, 0:DL],
                          scale=kvm[:, j:j + 1] if use_kvm else None)

            # ---- strips: Q projection + attention, interleaved ---------
            with tc.tile_pool(name="psQ", bufs=2, space="PSUM") as psQ, \
                 tc.tile_pool(name="psS", bufs=2, space="PSUM") as psS, \
                 tc.tile_pool(name="psC", bufs=2, space="PSUM") as psC:
                for sq in range(NSQ):
                    for mc in range(MC):
                        pq = psQ.tile([128, SQW], f32, tag="pq")
                        for kc in range(KC):
                            nc.tensor.matmul(
                                pq[:],
                                wqt[:, kc * DL + mc * 128:
                                    kc * DL + mc * 128 + 128],
                                xt[kc][:, ts(sq, SQW)],
                                start=(kc == 0), stop=(kc == KC - 1))
                        drain(qT[mc][:, ts(sq, SQW)], pq[:],
                              bias=bq2[:, mc:mc + 1] if has_bias else None)
                    for hp in range(MC):
                        pbs = [None] * skv
                        ct = psC.tile([128, SQW], f32, tag="ct")

                        def s_step(k):
                            spt = psS.tile([128, 1024], f32, tag="spt", name="spt")
                            nc.tensor.matmul(spt[:, 0:512],
                                             kT[hp][0:64, ts(k, 128)],
                                             qT[hp][0:64, ts(sq, SQW)],
                                             start=True, stop=True)
                            nc.tensor.matmul(spt[:, 512:1024],
                                             kT[hp][64:128, ts(k, 128)],
                                             qT[hp][64:128, ts(sq, SQW)],
                                             start=True, stop=True)
                            pb = pp.tile([128, 1024], fp16, tag="pb", name="pb")
                            drain(pb[:, 0:512], spt[:, 0:512])
                            drain(pb[:, 512:1024], spt[:, 512:1024])
                            pbs[k] = pb

                        def c_step(k):
                            for h in range(2):
                                nc.tensor.matmul(
                                    ct[h * 64:(h + 1) * 64, :],
                                    v_sb[:, k * DL + hp * 128 + h * 64:
                                         k * DL + hp * 128 + h * 64 + 64],
                                    pbs[k][:, h * 512:(h + 1) * 512],
                                    start=(k == 0), stop=(k == skv - 1),
                                    tile_position=(0, h * 64),
                                    skip_group_check=True)

                        s_step(0)
                        for k in range(1, skv):
                            s_step(k)
                            c_step(k - 1)
                        c_step(skv - 1)
                        stage = stg.tile([128, SQW], f32, tag="st")
                        drain(stage[:], ct[:])
                        nc.sync.dma_start(
                            OUT[hp * 128:(hp + 1) * 128, ts(sq, SQW)],
                            stage[:])

    nc.compile()
    return nc


def _get_nc(key):
    if key not in _cache:
        _cache[key] = _build(*key)
    return _cache[key]


def _make_in_maps(hidden_states, attention_mask, Wq, bq, Wk, bk, Wv, bv):
    hs = np.asarray(hidden_states, dtype=np.float32)
    am = np.asarray(attention_mask, dtype=np.float32)
    bq = np.asarray(bq, np.float32)
    bk = np.asarray(bk, np.float32)
    bv = np.asarray(bv, np.float32)
    has_bias = bool(bq.any() or bk.any() or bv.any())

    # hs.T per batch, fp16, feature-major
    xts = [np.ascontiguousarray(hs[b].T.astype(np.float16))
           for b in range(B)]

    # key compaction metadata per batch
    compact = True
    kvms, xkvts = [], []
    for b in range(B):
        valid = np.nonzero(am[b, 0, 0, :] >= 0)[0]
        if len(valid) > CAP:
            compact = False
            break
        xkvt = np.zeros((D, CAP), np.float16)
        xkvt[:, :len(valid)] = xts[b][:, valid]
        xkvts.append(np.ascontiguousarray(xkvt))
        kvm = np.zeros(CAP, np.float32)
        kvm[:len(valid)] = 1.0
        kvms.append(kvm)

    skv = (CAP if compact else S) // 128
    key = (skv, compact, has_bias)
    use_kvm = has_bias or not compact

    in_maps = []
    for c in range(N_CORES):
        b, g = divmod(c, 4)
        sl = slice(g * DL, (g + 1) * DL)
        m = {
            "xt": xts[b],
            "wqt": np.ascontiguousarray(
                np.asarray(Wq, np.float32)[sl, :].T.astype(np.float16)),
            "wkt": np.ascontiguousarray(
                np.asarray(Wk, np.float32)[sl, :].T.astype(np.float16)),
            "wvt": np.ascontiguousarray(
                np.asarray(Wv, np.float32)[sl, :].T.astype(np.float16)),
        }
        if compact:
            m["xkvt"] = xkvts[b]
        if has_bias:
            m["bq2"] = np.ascontiguousarray(bq[sl].reshape(MC, 128).T)
            m["bk2"] = np.ascontiguousarray(bk[sl].reshape(MC, 128).T)
            m["bv"] = np.ascontiguousarray(
                bv[sl].reshape(1, DL).astype(np.float16))
            m["ones"] = np.ones((1, 128), np.float16)
        if use_kvm:
            if compact:
                kvm2 = np.ascontiguousarray(kvms[b].reshape(skv, 128).T)
            else:
                kvm2 = np.ascontiguousarray(
                    (am[b, 0, 0, :] >= 0).astype(np.float32)
                    .reshape(skv, 128).T)
            m["kvm2"] = kvm2
        in_maps.append(m)
    return key, in_maps


def _gather(results):
    out = np.empty((B, S, D), np.float32)
    for c in range(N_CORES):
        b, g = divmod(c, 4)
        out[b, :, g * DL:(g + 1) * DL] = results[c]["out"].T
    return out


def run_sharded(key, in_maps, **kw):
    nc = _get_nc(key)
    return run_bass_kernel_spmd(nc, in_maps, core_ids=list(range(N_CORES)),
                                **kw)


def kernel(hidden_states, attention_mask, Wq, bq, Wk, bk, Wv, bv):
    key, in_maps = _make_in_maps(hidden_states, attention_mask,
                                 Wq, bq, Wk, bk, Wv, bv)
    res = run_sharded(key, in_maps)
    return _gather(res.results)
